# revision 22
# baseline (speedup 1.0000x reference)
"""AttentionBasedPruner Trainium2 kernel (single-pass, bf16-split, pipelined).

Per batch row: scores = gelu(x @ w1 + b1) @ w2; keep top-K (K=2867 of 4096)
tokens; emit kept token rows of x in ascending token order. Exact (rel 0.0).

Sharding: batch 32 -> 8 cores x 4 rows (RPC). Single pass over x:
  - stream x row-chunks [128, 4, 768] (token t = 128c + p), keep ALL 8 chunks
    of a row resident in SBUF until its scatter completes (bufs=12 pipeline).
  - PE-transpose 128x128 blocks into PSUM; split each transposed block into
    bf16 hi (ACT copy) + bf16 lo = residual (DVE sub); h = x@w1 computed as
    xb@wb + xl@wb + xb@wbl (three 1-cyc/row bf16 matmuls, exact to ~3e-6 --
    enough to reproduce the reference top-K selection exactly, verified 0
    mismatched rows). GELU+bias on ACT; scores via gt-as-stationary fp32
    matmuls giving s2d [128 p=t%128, 32 c=t/128] directly (no DRAM detour).
  - per-row threshold by lo/hi bisection on [-16,16], 32 fp32 steps; the
    chain for row r-1 is EMITTED interleaved into row r's chunk loop so the
    PE/DVE streams stay time-ordered (cross-partition count via ones-matmul;
    NOTE tensor_tensor_reduce crashes HW under axon -- use simple ops only).
  - kept mask -> output slots via ltri/ones matmul prefix sums -> per-block
    indirect-DMA scatters (multi-column offset APs misbehave on HW).
"""
import sys

sys.path.insert(0, "/opt/trn_rl_repo")
import numpy as np

B, N, D, H = 32, 4096, 768, 192
KEEP = int(N * 0.7)  # 2867
NCORES = 8
RPC = B // NCORES  # rows per core
MC = N // 512  # megachunks (512 tokens) per row
BIG = 3.0e7
BIS_LO, BIS_HI, BIS_ITERS = -16.0, 16.0, 32

_cache = {}


def _build():
    key = "nc"
    if key in _cache:
        return _cache[key]
    import concourse.bacc as bacc
    import concourse.tile as tile
    import concourse.mybir as mybir
    import concourse.bass as bass
    from concourse.masks import make_identity

    F32 = mybir.dt.float32
    BF16 = mybir.dt.bfloat16
    U8 = mybir.dt.uint8
    I32 = mybir.dt.int32
    GELU = mybir.ActivationFunctionType.Gelu
    ALU = mybir.AluOpType
    AX = mybir.AxisListType.X

    nc = bacc.Bacc(None, target_bir_lowering=False)
    X = nc.dram_tensor("x", [RPC, N, D], F32, kind="ExternalInput")
    W1 = nc.dram_tensor("w1", [D, H], F32, kind="ExternalInput")
    B1 = nc.dram_tensor("b1", [H], F32, kind="ExternalInput")
    W2 = nc.dram_tensor("w2", [H, 1], F32, kind="ExternalInput")
    Y = nc.dram_tensor("y", [RPC * KEEP, D], F32, kind="ExternalOutput")

    with tile.TileContext(nc) as tc:
        with (
            tc.tile_pool(name="const", bufs=1) as cp,
            tc.tile_pool(name="xin", bufs=12) as xpool,
            tc.tile_pool(name="xt", bufs=4) as xtpool,
            tc.tile_pool(name="gt", bufs=2) as gtpool,
            tc.tile_pool(name="row", bufs=1) as rowpool,
            tc.tile_pool(name="bis", bufs=2) as bp,
            tc.tile_pool(name="psT", bufs=2, space="PSUM") as psT,
            tc.tile_pool(name="psH", bufs=1, space="PSUM") as psH,
            tc.tile_pool(name="psW", bufs=1, space="PSUM") as psW,
            tc.tile_pool(name="psC", bufs=1, space="PSUM") as psC,
            tc.tile_pool(name="psP", bufs=1, space="PSUM") as psP,
        ):
            ident = cp.tile([128, 128], F32)
            make_identity(nc, ident[:])
            ones = cp.tile([128, 128], F32)
            nc.vector.memset(ones[:], 1.0)
            # ltri[k, m] = 1 iff k <= m  (inclusive prefix over partitions)
            ltri = cp.tile([128, 128], F32)
            nc.vector.memset(ltri[:], 1.0)
            nc.gpsimd.affine_select(
                out=ltri[:], in_=ltri[:], compare_op=ALU.is_ge, fill=0.0,
                base=0, pattern=[[1, 128]], channel_multiplier=-1,
            )
            w1s, wb, wbl = [], [], []
            for j in range(6):
                t = cp.tile([128, H], F32, tag=f"w1s{j}", name=f"w1s{j}")
                nc.sync.dma_start(t[:], W1[128 * j : 128 * (j + 1), :])
                w1s.append(t)
                tb = cp.tile([128, H], BF16, tag=f"wb{j}", name=f"wb{j}")
                nc.scalar.copy(tb[:], t[:])
                wb.append(tb)
                tl = cp.tile([128, H], BF16, tag=f"wbl{j}", name=f"wbl{j}")
                nc.vector.tensor_sub(tl[:], t[:], tb[:])
                wbl.append(tl)
            b1a = cp.tile([128, 1], F32)
            nc.sync.dma_start(b1a[:], B1[0:128, None])
            b1b = cp.tile([64, 1], F32)
            nc.sync.dma_start(b1b[:], B1[128:H, None])
            w2a = cp.tile([128, 1], F32)
            nc.sync.dma_start(w2a[:], W2[0:128, :])
            w2b = cp.tile([64, 1], F32)
            nc.sync.dma_start(w2b[:], W2[128:H, :])

            # per-row persistent tiles
            s2d = [rowpool.tile([128, 32], F32, tag=f"s2d{r}", name=f"s2d{r}")
                   for r in range(RPC)]
            lo = [rowpool.tile([128, 1], F32, tag=f"lo{r}", name=f"lo{r}")
                  for r in range(RPC)]
            hi = [rowpool.tile([128, 1], F32, tag=f"hi{r}", name=f"hi{r}")
                  for r in range(RPC)]
            didx = [rowpool.tile([128, 32], I32, tag=f"didx{r}", name=f"didx{r}")
                    for r in range(RPC)]
            xs_tiles = {}  # (r, mc) -> tile, kept resident until scattered

            def bis_steps(r, k0, k1):
                """Emit bisection iterations [k0, k1) for row r."""
                for k in range(k0, k1):
                    if k == 0:
                        nc.vector.memset(lo[r][:], BIS_LO)
                        nc.vector.memset(hi[r][:], BIS_HI)
                    mid = bp.tile([128, 1], F32, tag=f"mid{r}", name=f"mid{r}")
                    cmp = bp.tile([128, 32], F32, tag=f"cmp{r}", name=f"cmp{r}")
                    part = bp.tile([128, 1], F32, tag=f"part{r}", name=f"part{r}")
                    nc.vector.tensor_add(mid[:], lo[r][:], hi[r][:])
                    nc.vector.tensor_scalar_mul(mid[:], mid[:], 0.5)
                    nc.vector.tensor_scalar(
                        cmp[:], s2d[r][:], mid[:, 0:1], None, op0=ALU.is_ge
                    )
                    nc.vector.reduce_sum(part[:], cmp[:], axis=AX)
                    pcnt = psC.tile([128, 1], F32, tag="pcnt", name="pcnt")
                    nc.tensor.matmul(pcnt[:], ones[:], part[:], start=True, stop=True)
                    ge = bp.tile([128, 1], U8, tag=f"ge{r}", name=f"ge{r}")
                    lt = bp.tile([128, 1], U8, tag=f"lt{r}", name=f"lt{r}")
                    nc.vector.tensor_scalar(
                        ge[:], pcnt[:], float(KEEP), None, op0=ALU.is_ge
                    )
                    nc.vector.tensor_scalar(
                        lt[:], pcnt[:], float(KEEP), None, op0=ALU.is_lt
                    )
                    nc.vector.copy_predicated(lo[r][:], ge[:], mid[:])
                    nc.vector.copy_predicated(hi[r][:], lt[:], mid[:])

            def prep_scatter(r):
                """didx for row r from converged lo[r], then batched scatters."""
                kept = bp.tile([128, 32], F32, tag=f"kept{r}", name=f"kept{r}")
                nc.vector.tensor_scalar(
                    kept[:], s2d[r][:], lo[r][:, 0:1], None, op0=ALU.is_ge
                )
                pincl = psP.tile([128, 32], F32, tag="incl", name="incl")
                nc.tensor.matmul(pincl[:], ltri[:], kept[:], start=True, stop=True)
                pcols = psP.tile([128, 32], F32, tag="cols", name="cols")
                nc.tensor.matmul(pcols[:], ones[:], kept[:], start=True, stop=True)
                exA = bp.tile([128, 32], F32, tag="exA", name="exA")
                nc.vector.tensor_sub(exA[:], pincl[:], kept[:])
                cur = bp.tile([128, 32], F32, tag="scan0", name="scan0")
                nc.vector.memset(cur[:, 0:1], 0.0)
                nc.vector.tensor_copy(cur[:, 1:32], pcols[:, 0:31])
                for i, d in enumerate((1, 2, 4, 8, 16)):
                    nxt = bp.tile([128, 32], F32, tag=f"scan{1 - i % 2}",
                                  name=f"scan{1 - i % 2}")
                    nc.vector.tensor_copy(nxt[:, 0:d], cur[:, 0:d])
                    nc.vector.tensor_add(
                        nxt[:, d:32], cur[:, d:32], cur[:, 0 : 32 - d]
                    )
                    cur = nxt
                slot = bp.tile([128, 32], F32, tag="slot", name="slot")
                nc.vector.tensor_add(slot[:], exA[:], cur[:])
                nc.vector.tensor_scalar_add(slot[:], slot[:], float(r * KEEP))
                keptu = bp.tile([128, 32], U8, tag="keptu", name="keptu")
                nc.vector.tensor_copy(keptu[:], kept[:])
                slotf = bp.tile([128, 32], F32, tag="slotf", name="slotf")
                nc.vector.memset(slotf[:], BIG)
                nc.vector.copy_predicated(slotf[:], keptu[:], slot[:])
                nc.vector.tensor_copy(didx[r][:], slotf[:])
                # indirect scatters from the resident chunks (one per
                # 128-token block: multi-column offsets misbehave on HW)
                for mc in range(MC):
                    xs = xs_tiles.pop((r, mc))
                    for g in range(4):
                        c = 4 * mc + g
                        nc.gpsimd.indirect_dma_start(
                            out=Y[:, :],
                            out_offset=bass.IndirectOffsetOnAxis(
                                ap=didx[r][:, c : c + 1], axis=0
                            ),
                            in_=xs[:, g, :],
                            in_offset=None,
                            bounds_check=(r + 1) * KEEP - 1,
                            oob_is_err=False,
                        )

            # bisection iterations of row r-1 spread over row r's 8 chunks
            def bis_share(m):
                a = (BIS_ITERS * m + 7) // 8
                b = (BIS_ITERS * (m + 1) + 7) // 8
                return a, min(b, BIS_ITERS)

            for r in range(RPC):
                for mc in range(MC):
                    xs = xpool.tile([128, 4, D], F32, tag="xs",
                                    name=f"xs{r}_{mc}")
                    xs_tiles[(r, mc)] = xs
                    nc.sync.dma_start(
                        xs[:],
                        X[r, mc * 512 : (mc + 1) * 512, :].rearrange(
                            "(g p) d -> p g d", p=128
                        ),
                    )
                    ph = psH.tile([128, 1024], F32, tag="ph", name="ph")
                    for j in range(6):
                        pt = psT.tile([128, 512], F32, tag="pt", name="pt")
                        for g in range(4):
                            nc.tensor.transpose(
                                pt[:, 128 * g : 128 * (g + 1)],
                                xs[:, g, 128 * j : 128 * (j + 1)],
                                ident[:],
                            )
                        # bf16 split of the transposed block: xtb = bf16(x),
                        # xtl = bf16(x - xtb); ACT rounds, DVE subtracts
                        # (Pool can't read PSUM)
                        xtb = xtpool.tile([128, 512], BF16, tag="xtb",
                                          name="xtb")
                        nc.scalar.copy(xtb[:], pt[:])
                        xtl = xtpool.tile([128, 512], BF16, tag="xtl",
                                          name="xtl")
                        nc.vector.tensor_sub(xtl[:], pt[:], xtb[:])
                        # x@w1 = xtb@wb + xtl@wb + xtb@wbl (exact to ~1e-6);
                        # xtl consumers go LAST so the ACT->DVE split chain
                        # overlaps the first four matmuls
                        first, last = (j == 0), (j == 5)
                        nc.tensor.matmul(
                            ph[:, 0:512], wb[j][:, 0:128], xtb[:],
                            start=first, stop=False,
                        )
                        nc.tensor.matmul(
                            ph[:, 0:512], wbl[j][:, 0:128], xtb[:],
                            start=False, stop=False,
                        )
                        nc.tensor.matmul(
                            ph[0:64, 512:1024], wb[j][:, 128:H], xtb[:],
                            start=first, stop=False,
                        )
                        nc.tensor.matmul(
                            ph[0:64, 512:1024], wbl[j][:, 128:H], xtb[:],
                            start=False, stop=False,
                        )
                        nc.tensor.matmul(
                            ph[:, 0:512], wb[j][:, 0:128], xtl[:],
                            start=False, stop=last,
                        )
                        nc.tensor.matmul(
                            ph[0:64, 512:1024], wb[j][:, 128:H], xtl[:],
                            start=False, stop=last,
                        )
                    gt0 = gtpool.tile([128, 512], F32, tag="gt0", name="gt0")
                    gt1 = gtpool.tile([64, 512], F32, tag="gt1", name="gt1")
                    nc.scalar.activation(
                        gt0[:], ph[:, 0:512], GELU, bias=b1a[:, 0:1]
                    )
                    nc.scalar.activation(
                        gt1[:], ph[0:64, 512:1024], GELU, bias=b1b[:, 0:1]
                    )
                    # w2 matmuls: gt as stationary -> scores land [t%128, c]
                    psw = psW.tile([128, 4], F32, tag="psw", name="psw")
                    for g in range(4):
                        nc.tensor.matmul(
                            psw[:, g : g + 1],
                            gt0[:, 128 * g : 128 * (g + 1)], w2a[:],
                            start=True, stop=False,
                        )
                        nc.tensor.matmul(
                            psw[:, g : g + 1],
                            gt1[:, 128 * g : 128 * (g + 1)], w2b[:],
                            start=False, stop=True,
                        )
                    nc.vector.tensor_copy(
                        s2d[r][:, 4 * mc : 4 * mc + 4], psw[:]
                    )
                    # interleave previous row's bisection chain
                    if r > 0:
                        k0, k1 = bis_share(mc)
                        bis_steps(r - 1, k0, k1)
                if r > 0:
                    prep_scatter(r - 1)
            # tail: last row's chain + scatter
            bis_steps(RPC - 1, 0, BIS_ITERS)
            prep_scatter(RPC - 1)

    nc.compile()
    _cache[key] = nc
    return nc


def kernel(x, w1, b1, w2, b2=None, trace=False):
    from concourse.bass_utils import run_bass_kernel_spmd

    nc = _build()
    x = np.ascontiguousarray(np.asarray(x, dtype=np.float32))
    w1 = np.ascontiguousarray(np.asarray(w1, dtype=np.float32))
    b1 = np.ascontiguousarray(np.asarray(b1, dtype=np.float32))
    w2 = np.ascontiguousarray(np.asarray(w2, dtype=np.float32))
    in_maps = [
        {
            "x": np.ascontiguousarray(x[c * RPC : (c + 1) * RPC]),
            "w1": w1,
            "b1": b1,
            "w2": w2,
        }
        for c in range(NCORES)
    ]
    res = run_bass_kernel_spmd(
        nc, in_maps, core_ids=list(range(NCORES)), trace=trace
    )
    out = np.concatenate(
        [res.results[c]["y"].reshape(RPC, KEEP, D) for c in range(NCORES)], axis=0
    )
    if trace:
        return out, res
    return out
